# revision 3
# baseline (speedup 1.0000x reference)
"""Single-head causal self-attention on 8 Trainium2 NeuronCores.

Problem: x[B=8, T=2048, D=2048], Wq/Wk/Wv[D, 128], bq/bk/bv[128]
  q,k,v = x @ W* + b*        (per batch)
  att   = softmax(mask(q k^T / sqrt(128)))
  out   = att @ v            -> [B, T, 128]

Sharding: data-parallel over batch; core b processes batch element b.
All matmuls run in float32r (tf32-like, 1 cyc/row at N>=256) with fp32
PSUM accumulation.

Per-core layout strategy:
  phase 1: load x naturally, PE-transpose to xT (D on partitions),
           Q^T,K^T = [H,T] via matmul(lhsT=W tile, rhs=xT tile),
           V^T likewise then PE-transposed to natural V [T,H].
  phase 2: per 512-wide q-range j: for each k-tile kt<=diag:
           S^T[k,q] = matmul(lhsT=K^T slice, rhs=Q^T range)   (contract H)
           causal mask: add -1e4 on diagonal tiles, exp via ACT with
           fused 1/sqrt(H) scale -> P^T (fp32r),
           O^T += matmul(lhsT=V tile, rhs=P^T)
           rowsum += matmul(lhsT=ones[128,128], rhs=P^T)  (bcast rows)
           then O = (O^T * recip(rowsum)) transposed back, stored.
"""
from contextlib import ExitStack

import numpy as np

import concourse.bacc as bacc
import concourse.bass as bass
import concourse.mybir as mybir
import concourse.tile as tile
from concourse.masks import make_identity
from concourse.bass_utils import run_bass_kernel_spmd

B, T, D, H = 8, 2048, 2048, 128
KT = D // 128          # 16 contraction k-tiles for the projections
QR = 512               # q-range width (free dim of attention matmuls)
NJ = T // QR           # 4 q-ranges
TCH = 512              # t-chunk width in phase 1
NCH = T // TCH         # 4 t-chunks
SCALE = 1.0 / np.sqrt(np.float32(H))
MASK_NEG = -1.0e4

FP32 = mybir.dt.float32
FP32R = mybir.dt.float32r
AF = mybir.ActivationFunctionType

_CACHE = {}


def build():
    nc = bacc.Bacc()
    x = nc.declare_dram_parameter("x", [T, D], FP32, isOutput=False)
    wq = nc.declare_dram_parameter("wq", [D, H], FP32, isOutput=False)
    wk = nc.declare_dram_parameter("wk", [D, H], FP32, isOutput=False)
    wv = nc.declare_dram_parameter("wv", [D, H], FP32, isOutput=False)
    bq = nc.declare_dram_parameter("bq", [H, 1], FP32, isOutput=False)
    bk = nc.declare_dram_parameter("bk", [H, 1], FP32, isOutput=False)
    bv = nc.declare_dram_parameter("bv", [H, 1], FP32, isOutput=False)
    out = nc.declare_dram_parameter("out", [T, H], FP32, isOutput=True)

    with tile.TileContext(nc) as tc, ExitStack() as octx:
        persist = octx.enter_context(tc.tile_pool(name="persist", bufs=1))

        # ---- constants -------------------------------------------------
        ident = persist.tile([128, 128], FP32, tag="ident")
        make_identity(nc, ident[:])

        ones_f = persist.tile([128, 128], FP32, tag="ones_f")
        nc.gpsimd.memset(ones_f[:], 1.0)
        ones_r = persist.tile([128, 128], FP32R, tag="ones_r")
        nc.vector.tensor_copy(ones_r[:], ones_f[:])

        # diagonal causal masks: mneg[i][k, q'] = 0 where q' >= k + 128*i
        # else MASK_NEG (added to raw scores before exp)
        mneg = []
        for i in range(4):
            m = persist.tile([128, QR], FP32, tag=f"mneg{i}")
            nc.gpsimd.memset(m[:], 0.0)
            nc.gpsimd.affine_select(
                out=m[:], in_=m[:],
                compare_op=mybir.AluOpType.is_ge,
                fill=MASK_NEG,
                base=-128 * i,
                pattern=[[1, QR]],        # + q'
                channel_multiplier=-1,    # - k  => q' - k - 128i >= 0 -> keep 0
            )
            mneg.append(m)

        # ---- weights + biases -----------------------------------------
        w_r = {}
        with tc.tile_pool(name="wtmp", bufs=2) as wtmp:
            for name, wd in (("q", wq), ("k", wk), ("v", wv)):
                wf = wtmp.tile([128, D], FP32, tag="wf")
                # [D, H] -> SBUF [128(d%128), KT*H] with free = (d//128, h)
                nc.sync.dma_start(
                    wf[:].rearrange("p (kt h) -> p kt h", kt=KT),
                    wd[:].rearrange("(kt p) h -> p kt h", p=128))
                wr = persist.tile([128, D], FP32R, tag=f"w_{name}")
                nc.vector.tensor_copy(wr[:], wf[:])
                w_r[name] = wr

        b_sb = {}
        for name, bd in (("q", bq), ("k", bk), ("v", bv)):
            t_ = persist.tile([128, 1], FP32, tag=f"b_{name}")
            nc.sync.dma_start(t_[:], bd[:])
            b_sb[name] = t_

        # ---- persistent activations -----------------------------------
        qt_sb = persist.tile([128, T], FP32R, tag="qt")   # Q^T [h, t]
        kt_sb = persist.tile([128, T], FP32R, tag="kt")   # K^T [h, t]
        v_nat = [persist.tile([128, H], FP32R, tag=f"v{i}", name=f"v_nat{i}")
                 for i in range(KT)]

        # ================= phase 1: projections ========================
        with ExitStack() as ctx:
            xnat = ctx.enter_context(tc.tile_pool(name="xnat", bufs=8))
            xt_pool = ctx.enter_context(tc.tile_pool(name="xt", bufs=3))
            vt_pool = ctx.enter_context(tc.tile_pool(name="vt", bufs=2))
            ps_xt = ctx.enter_context(
                tc.tile_pool(name="ps_xt", bufs=2, space="PSUM"))
            ps_acc = ctx.enter_context(
                tc.tile_pool(name="ps_acc", bufs=1, space="PSUM"))
            ps_vt = ctx.enter_context(
                tc.tile_pool(name="ps_vt", bufs=1, space="PSUM"))

            for c in range(NCH):
                xs = []
                for tb in range(TCH // 128):
                    xt_ = xnat.tile([128, D], FP32, tag="xnat")
                    r0 = c * TCH + tb * 128
                    nc.sync.dma_start(xt_[:], x[r0:r0 + 128, :])
                    xs.append(xt_)

                q_ps = ps_acc.tile([128, TCH], FP32, tag="q_ps")
                k_ps = ps_acc.tile([128, TCH], FP32, tag="k_ps")
                v_ps = ps_acc.tile([128, TCH], FP32, tag="v_ps")

                xt_sb = [None] * KT

                def emit_xt(kt):
                    xt_ps = ps_xt.tile([128, TCH], FP32, tag="xt_ps")
                    for tb in range(TCH // 128):
                        nc.tensor.transpose(
                            xt_ps[:, tb * 128:(tb + 1) * 128],
                            xs[tb][:, kt * 128:(kt + 1) * 128],
                            ident[:])
                    t_ = xt_pool.tile([128, TCH], FP32R, tag="xt_sb")
                    nc.vector.tensor_copy(t_[:], xt_ps[:])
                    xt_sb[kt] = t_

                emit_xt(0)
                for kt in range(KT):
                    if kt + 1 < KT:
                        emit_xt(kt + 1)
                    st, sp = kt == 0, kt == KT - 1
                    nc.tensor.matmul(
                        q_ps[:], w_r["q"][:, kt * 128:(kt + 1) * 128],
                        xt_sb[kt][:], start=st, stop=sp)
                    nc.tensor.matmul(
                        k_ps[:], w_r["k"][:, kt * 128:(kt + 1) * 128],
                        xt_sb[kt][:], start=st, stop=sp)
                    nc.tensor.matmul(
                        v_ps[:], w_r["v"][:, kt * 128:(kt + 1) * 128],
                        xt_sb[kt][:], start=st, stop=sp)
                    xt_sb[kt] = None

                c0 = c * TCH
                nc.scalar.activation(qt_sb[:, c0:c0 + TCH], q_ps[:],
                                     AF.Identity, bias=b_sb["q"][:])
                nc.scalar.activation(kt_sb[:, c0:c0 + TCH], k_ps[:],
                                     AF.Identity, bias=b_sb["k"][:])
                vt_sb = vt_pool.tile([128, TCH], FP32, tag="vt_sb")
                nc.scalar.activation(vt_sb[:], v_ps[:],
                                     AF.Identity, bias=b_sb["v"][:])
                for tb in range(TCH // 128):
                    vt_ps = ps_vt.tile([128, H], FP32, tag="vt_ps")
                    nc.tensor.transpose(
                        vt_ps[:], vt_sb[:, tb * 128:(tb + 1) * 128], ident[:])
                    nc.vector.tensor_copy(
                        v_nat[c * (TCH // 128) + tb][:], vt_ps[:])

        # ================= phase 2: attention ==========================
        with ExitStack() as ctx:
            pp = ctx.enter_context(tc.tile_pool(name="pp", bufs=4))
            on_pool = ctx.enter_context(tc.tile_pool(name="on", bufs=2))
            os_pool = ctx.enter_context(tc.tile_pool(name="os", bufs=2))
            ps_s = ctx.enter_context(
                tc.tile_pool(name="ps_s", bufs=3, space="PSUM"))
            ps_o = ctx.enter_context(
                tc.tile_pool(name="ps_o", bufs=2, space="PSUM"))
            ps_r = ctx.enter_context(
                tc.tile_pool(name="ps_r", bufs=1, space="PSUM"))
            ps_ot = ctx.enter_context(
                tc.tile_pool(name="ps_ot", bufs=1, space="PSUM"))

            LOOK = 2
            for j in range(NJ):
                kmax = 4 * j + 4
                q0 = j * QR
                o_ps = ps_o.tile([128, QR], FP32, tag="o_ps")
                r_ps = ps_r.tile([128, QR], FP32, tag="r_ps")
                p_sb = [None] * kmax

                def emit_s(kt):
                    s_ps = ps_s.tile([128, QR], FP32, tag="s_ps")
                    nc.tensor.matmul(
                        s_ps[:], kt_sb[:, kt * 128:(kt + 1) * 128],
                        qt_sb[:, q0:q0 + QR], start=True, stop=True)
                    i = kt - 4 * j
                    if i >= 0:
                        nc.vector.tensor_add(s_ps[:], s_ps[:], mneg[i][:])
                    p = pp.tile([128, QR], FP32R, tag="p")
                    nc.scalar.activation(p[:], s_ps[:], AF.Exp, scale=SCALE)
                    p_sb[kt] = p

                for kt in range(min(LOOK, kmax)):
                    emit_s(kt)
                for kt in range(kmax):
                    if kt + LOOK < kmax:
                        emit_s(kt + LOOK)
                    st, sp = kt == 0, kt == kmax - 1
                    nc.tensor.matmul(o_ps[:], v_nat[kt][:], p_sb[kt][:],
                                     start=st, stop=sp)
                    nc.tensor.matmul(r_ps[:], ones_r[:], p_sb[kt][:],
                                     start=st, stop=sp)
                    p_sb[kt] = None

                recip = on_pool.tile([128, QR], FP32, tag="recip")
                nc.vector.reciprocal(recip[:], r_ps[:])
                onorm = on_pool.tile([128, QR], FP32, tag="onorm")
                nc.vector.tensor_mul(onorm[:], o_ps[:], recip[:])
                for i in range(QR // 128):
                    ot_ps = ps_ot.tile([128, H], FP32, tag="ot_ps")
                    nc.tensor.transpose(
                        ot_ps[:], onorm[:, i * 128:(i + 1) * 128], ident[:])
                    osb = os_pool.tile([128, H], FP32, tag="osb")
                    nc.scalar.copy(osb[:], ot_ps[:])
                    r0 = q0 + i * 128
                    nc.sync.dma_start(out[r0:r0 + 128, :], osb[:])

    nc.finalize()
    return nc


def _get_nc():
    if "nc" not in _CACHE:
        _CACHE["nc"] = build()
    return _CACHE["nc"]


def kernel(x, Wq, bq, Wk, bk, Wv, bv, _trace=False):
    x = np.ascontiguousarray(np.asarray(x, dtype=np.float32))
    in_common = {
        "wq": np.ascontiguousarray(np.asarray(Wq, np.float32)),
        "wk": np.ascontiguousarray(np.asarray(Wk, np.float32)),
        "wv": np.ascontiguousarray(np.asarray(Wv, np.float32)),
        "bq": np.ascontiguousarray(np.asarray(bq, np.float32).reshape(H, 1)),
        "bk": np.ascontiguousarray(np.asarray(bk, np.float32).reshape(H, 1)),
        "bv": np.ascontiguousarray(np.asarray(bv, np.float32).reshape(H, 1)),
    }
    nc = _get_nc()
    in_maps = [dict(in_common, x=np.ascontiguousarray(x[b])) for b in range(B)]
    res = run_bass_kernel_spmd(nc, in_maps, core_ids=list(range(B)),
                               trace=_trace)
    out = np.stack([res.results[b]["out"] for b in range(B)], axis=0)
    if _trace:
        _CACHE["last_exec_time_ns"] = res.exec_time_ns
        _CACHE["last_results"] = res
    return out
